# revision 2
# baseline (speedup 1.0000x reference)
"""Trainium2 Bass kernel v3 for nn_CrossGraphConvolution (hardware-loop design).

Backend model (measured on this setup): per-call cost is dominated by
per-STATIC-instruction overhead (~50-130us each, NEFF translation), while
dynamic execution runs at silicon speed. So the kernel is a ~150-static-
instruction body inside a For_i hardware loop over 8 m-windows of 512.

Math (per batch b, one NeuronCore each):
    S^T[n,m] = xn[:,n] . gn[:,m]        (cosine similarity, transposed)
    P^T = exp(S^T)                       (softmax numerator; max-subtract
                                          skipped: cosines are in [-1,1])
    o3[o,m] = sum_n xw[n,o] P^T[n,m]     (aggregation pre-projected by W,
                                          fp8 DoubleRow: 2 n-chunks/matmul)
    rows[m] = sum_n P^T[n,m]             (softmax denominator,
                                          gpsimd partition_all_reduce)
    y[o,m]  = LeakyReLU(o3)/rows * a + b (LeakyReLU commutes with the
                                          positive 1/rows scale; BN folded)

Host precomputes xn, gn (l2-normalized bf16), xw = (x^T W) in
[n-chunk-partition, o] layout (fp8e4), and BN a/b.
"""

import sys

import numpy as np

if "/opt/trn_rl_repo" not in sys.path:
    sys.path.insert(0, "/opt/trn_rl_repo")

B, C, N, M, OUT = 8, 128, 4096, 4096, 128
NJ = N // 128           # 32 n-chunks
MW = 512                # m-window width
NMW = M // MW           # 8 m-windows
EPS_BN = 1e-5
NEG_SLOPE = 0.01


def _apply_bir_passes():
    """Ldweights dedup + single-wait legalization (same as baseline)."""
    import json

    import concourse.bass as bass

    if getattr(bass.Bass, "_bir_passes_applied", False):
        return
    orig = bass.Bass.to_json_bytes

    def patched(self):
        bir = json.loads(orig(self))
        for fn in bir.get("functions", []):
            for blk in fn.get("blocks", []):
                insts = blk.get("instructions", [])
                last_ldw = {}
                kept = []
                for ins in insts:
                    if ins.get("opcode") == "Ldweights":
                        eng = ins.get("engine")
                        key = json.dumps(
                            [
                                ins.get("ins"),
                                ins.get("perf_mode"),
                                ins.get("is_transpose"),
                                ins.get("tile_position"),
                            ],
                            sort_keys=True,
                        )
                        ow = (ins.get("sync_info") or {}).get("on_wait") or []
                        upd = (ins.get("sync_info") or {}).get("on_update") or []
                        if last_ldw.get(eng) == key and not upd:
                            if ow:
                                kept.append(
                                    {
                                        "debug": ins.get("debug", 0),
                                        "engine": eng,
                                        "ins": [],
                                        "name": ins["name"] + "-dedup",
                                        "opcode": "NoOp",
                                        "outs": [],
                                        "sync_info": {
                                            "on_update": [],
                                            "on_wait": ow,
                                        },
                                    }
                                )
                            continue
                        last_ldw[eng] = key
                    kept.append(ins)
                new_insts = []
                for ins in kept:
                    si = ins.get("sync_info")
                    ow = (si or {}).get("on_wait") or []
                    if len(ow) > 1:
                        for k, w in enumerate(ow[:-1]):
                            new_insts.append(
                                {
                                    "debug": ins.get("debug", 0),
                                    "engine": ins["engine"],
                                    "ins": [],
                                    "name": f"{ins['name']}-w{k}",
                                    "opcode": "NoOp",
                                    "outs": [],
                                    "sync_info": {
                                        "on_update": [],
                                        "on_wait": [w],
                                    },
                                }
                            )
                        si["on_wait"] = [ow[-1]]
                    new_insts.append(ins)
                blk["instructions"] = new_insts
        return json.dumps(bir).encode()

    bass.Bass.to_json_bytes = patched
    bass.Bass._bir_passes_applied = True


def _bcast(ap, parts):
    """Partition-stride-0 view of a [1, ...] DRAM AP, for DMA broadcast."""
    import concourse.bass as bass

    return bass.AP(
        tensor=ap.tensor,
        offset=ap.offset,
        ap=[[0, parts]] + [list(d) for d in ap.ap[1:]],
    )


def _ap3(sl, t_stride, t_n, m_stride, m_n):
    """3D AP view [partition][t][m] of a 2D tile slice (for DoubleRow)."""
    import concourse.bass as bass

    return bass.AP(
        tensor=sl.tensor,
        offset=sl.offset,
        ap=[list(sl.ap[0]), [t_stride, t_n], [m_stride, m_n]],
    )


def build_nc(repeats: int = 1, o3_mode: str = "dr", lrelu_mode: str = "act"):
    import concourse.bass as bass
    import concourse.tile as tile
    from concourse import bass_isa, mybir

    _apply_bir_passes()

    f32 = mybir.dt.float32
    bf16 = mybir.dt.bfloat16
    f8 = mybir.dt.float8e4
    ALU = mybir.AluOpType
    ACTF = mybir.ActivationFunctionType
    dr = o3_mode == "dr"
    xw_dt = f8 if dr else bf16
    pt_dt = f8 if dr else bf16

    nc = bass.Bass("TRN2")
    xn_d = nc.dram_tensor("xn", [C, N], bf16, kind="ExternalInput")
    gn_d = nc.dram_tensor("gn", [C, M], bf16, kind="ExternalInput")
    xw_d = nc.dram_tensor("xw", [128, NJ * OUT], xw_dt, kind="ExternalInput")
    ab_d = nc.dram_tensor("ab", [OUT, 2], f32, kind="ExternalInput")
    y_d = nc.dram_tensor("y", [OUT, M], bf16, kind="ExternalOutput")

    with tile.TileContext(nc) as tc:
        with (
            tc.tile_pool(name="const", bufs=1) as const,
            tc.tile_pool(name="sb", bufs=1) as sb,
            tc.tile_pool(name="rws", bufs=2) as rws,
            tc.tile_pool(name="ep", bufs=2) as ep,
            tc.tile_pool(name="drp", bufs=2, space="DRAM") as drp,
            tc.tile_pool(name="stp", bufs=2, space="PSUM") as stp,
            tc.tile_pool(name="o3p", bufs=1, space="PSUM") as o3p,
            tc.tile_pool(name="rwp", bufs=1, space="PSUM") as rwp,
            tc.tile_pool(name="bcp", bufs=1, space="PSUM") as bcp,
        ):
            ab_sb = const.tile([OUT, 2], f32, tag="ab", name="ab_sb")
            nc.gpsimd.dma_start(out=ab_sb, in_=ab_d[:])
            ones_st = const.tile([128, 2], f8 if dr else bf16, tag="ones", name="ones_st")
            nc.vector.memset(ones_st, 1.0)
            ones_bc = const.tile([1, 128], f32, tag="onesbc", name="ones_bc")
            nc.vector.memset(ones_bc, 1.0)
            xn_sb = sb.tile([C, N], bf16, tag="xn", name="xn_sb")
            gn_sb = sb.tile([C, M], bf16, tag="gn", name="gn_sb")
            xw_sb = sb.tile([128, NJ * OUT], xw_dt, tag="xw", name="xw_sb")
            nc.gpsimd.dma_start(out=xn_sb, in_=xn_d[:])
            nc.gpsimd.dma_start(out=gn_sb, in_=gn_d[:])
            nc.gpsimd.dma_start(out=xw_sb, in_=xw_d[:])

            # output staging (dynamic DMA offsets don't compile; DVE writes
            # the window, one static DMA ships the full tensor at the end)
            y_all = sb.tile([OUT, M], bf16, tag="yall", name="y_all")
            # P^T for one m-window, n-chunk-major: pt[:, nj*MW + m]
            pt = sb.tile([128, NJ * MW], pt_dt, tag="pt", name="pt")

            def window_body(mw):
                o3 = o3p.tile([OUT, MW], f32, tag="o3", name="o3")
                # stage the m-window of gn once: keeps the 32 S^T matmuls on
                # static APs (dynamic APs exhaust PE offset registers)
                gwin = ep.tile([C, MW], bf16, tag="gwin", name="gwin")
                nc.vector.tensor_copy(gwin, gn_sb[:, bass.ts(mw, MW)])
                for pj in range(NJ // 2):
                    for t in range(2):
                        nj = 2 * pj + t
                        st = stp.tile([128, MW], f32, tag="st", name="st")
                        nc.tensor.matmul(
                            st,
                            xn_sb[:, nj * 128 : (nj + 1) * 128],
                            gwin,
                            start=True,
                            stop=True,
                        )
                        nc.scalar.activation(
                            out=pt[:, nj * MW : (nj + 1) * MW],
                            in_=st,
                            func=ACTF.Exp,
                        )
                    if dr:
                        nc.tensor.matmul(
                            o3,
                            _ap3(
                                xw_sb[:, pj * 256 : (pj + 1) * 256],
                                128, 2, 1, 128,
                            ),
                            _ap3(pt[:, 2 * pj * MW :], MW, 2, 1, MW),
                            start=(pj == 0),
                            stop=(pj == NJ // 2 - 1),
                            perf_mode=mybir.MatmulPerfMode.DoubleRow,
                        )
                    else:
                        for t in range(2):
                            nj = 2 * pj + t
                            nc.tensor.matmul(
                                o3,
                                xw_sb[:, nj * 128 : (nj + 1) * 128],
                                pt[:, nj * MW : (nj + 1) * MW],
                                start=(nj == 0),
                                stop=(nj == NJ - 1),
                            )

                # softmax denominator: ones-stationary matmuls over P^T
                # (consecutive, so the ones Ldweights dedups to one; DoubleRow
                # with a 1-row output miscompiles, so plain mode here)
                rowsum = rwp.tile([1, MW], f32, tag="rw", name="rowsum")
                for nj in range(NJ):
                    nc.tensor.matmul(
                        rowsum,
                        ones_st[:, 0:1],
                        pt[:, nj * MW : (nj + 1) * MW],
                        start=(nj == 0),
                        stop=(nj == NJ - 1),
                    )
                rr1 = rws.tile([1, MW], f32, tag="rr1", name="rr1")
                nc.vector.reciprocal(out=rr1, in_=rowsum)
                # broadcast to 128 partitions via a K=1 matmul (DMA inside a
                # For_i body goes through the TriggerDma ISA path, which
                # miscompiles on this toolchain)
                rr = bcp.tile([128, MW], f32, tag="rr", name="rr")
                nc.tensor.matmul(
                    rr, ones_bc[0:1, :], rr1, start=True, stop=True
                )

                # epilogue: LeakyReLU, /rows, BN affine
                z = ep.tile([OUT, MW], f32, tag="z", name="z")
                if lrelu_mode == "act":
                    nc.scalar.activation(
                        out=z, in_=o3, func=ACTF.Lrelu, alpha=NEG_SLOPE
                    )
                else:
                    zt = ep.tile([OUT, MW], f32, tag="zt", name="zt")
                    nc.vector.tensor_scalar(
                        out=zt, in0=o3, scalar1=NEG_SLOPE, scalar2=None,
                        op0=ALU.mult,
                    )
                    nc.vector.tensor_tensor(out=z, in0=o3, in1=zt, op=ALU.max)
                z2 = ep.tile([OUT, MW], f32, tag="z2", name="z2")
                nc.vector.tensor_tensor(out=z2, in0=z, in1=rr, op=ALU.mult)
                nc.vector.tensor_scalar(
                    out=y_all[:, bass.ts(mw, MW)],
                    in0=z2,
                    scalar1=ab_sb[:, 0:1],
                    scalar2=ab_sb[:, 1:2],
                    op0=ALU.mult,
                    op1=ALU.add,
                )

            if repeats == 1:
                with tc.For_i(0, NMW, 1) as mw:
                    window_body(mw)
            else:
                with tc.For_i(0, repeats, 1):
                    with tc.For_i(0, NMW, 1) as mw:
                        window_body(mw)
            nc.gpsimd.dma_start(out=y_d[:], in_=y_all)
    return nc


_nc_cache: dict = {}


def _prep(input, target_g, weight, gamma, beta, running_mean, running_var):
    import ml_dtypes

    x = np.asarray(input, dtype=np.float32)
    g = np.asarray(target_g, dtype=np.float32)
    w = np.asarray(weight, dtype=np.float32)
    gamma = np.asarray(gamma, dtype=np.float32).reshape(OUT)
    beta = np.asarray(beta, dtype=np.float32).reshape(OUT)
    mean = np.asarray(running_mean, dtype=np.float32).reshape(OUT)
    var = np.asarray(running_var, dtype=np.float32).reshape(OUT)

    a_sc = (gamma / np.sqrt(var + EPS_BN)).astype(np.float32)
    b_sc = (beta - mean * a_sc).astype(np.float32)
    ab = np.ascontiguousarray(np.stack([a_sc, b_sc], axis=1))

    xn = x / np.maximum(np.sqrt((x * x).sum(axis=1, keepdims=True)), 1e-12)
    gn = g / np.maximum(np.sqrt((g * g).sum(axis=1, keepdims=True)), 1e-12)
    xn16 = np.ascontiguousarray(xn.astype(ml_dtypes.bfloat16))
    gn16 = np.ascontiguousarray(gn.astype(ml_dtypes.bfloat16))

    # xw[b, p, nj*128+o] = (x[b]^T @ W)[nj*128+p, o]
    xw = np.einsum("bcn,co->bno", x, w)
    xw = xw.reshape(B, NJ, 128, OUT).transpose(0, 2, 1, 3).reshape(B, 128, NJ * OUT)
    xw8 = np.ascontiguousarray(
        np.clip(xw, -224.0, 224.0).astype(ml_dtypes.float8_e4m3)
    )
    return [
        {"xn": xn16[b], "gn": gn16[b], "xw": xw8[b], "ab": ab} for b in range(B)
    ]


def kernel(input, target_g, weight, gamma, beta, running_mean, running_var):
    from concourse.bass_utils import run_bass_kernel_spmd

    if "nc" not in _nc_cache:
        _nc_cache["nc"] = build_nc(repeats=1)
    nc = _nc_cache["nc"]
    in_maps = _prep(
        input, target_g, weight, gamma, beta, running_mean, running_var
    )
    res = run_bass_kernel_spmd(nc, in_maps, core_ids=list(range(B)))
    return np.stack([res.results[b]["y"] for b in range(B)]).astype(np.float32)


# revision 4
# speedup vs baseline: 1.2342x; 1.2342x over previous
"""Trainium2 Bass kernel v3 for nn_CrossGraphConvolution (hardware-loop design).

Backend model (measured on this setup): per-call cost is dominated by
per-STATIC-instruction overhead (~50-130us each, NEFF translation), while
dynamic execution runs at silicon speed. So the kernel is a ~150-static-
instruction body inside a For_i hardware loop over 8 m-windows of 512.

Math (per batch b, one NeuronCore each):
    S^T[n,m] = xn[:,n] . gn[:,m]        (cosine similarity, transposed)
    P^T = exp(S^T)                       (softmax numerator; max-subtract
                                          skipped: cosines are in [-1,1])
    o3[o,m] = sum_n xw[n,o] P^T[n,m]     (aggregation pre-projected by W,
                                          fp8 DoubleRow: 2 n-chunks/matmul)
    rows[m] = sum_n P^T[n,m]             (softmax denominator,
                                          gpsimd partition_all_reduce)
    y[o,m]  = LeakyReLU(o3)/rows * a + b (LeakyReLU commutes with the
                                          positive 1/rows scale; BN folded)

Host precomputes xn, gn (l2-normalized bf16), xw = (x^T W) in
[n-chunk-partition, o] layout (fp8e4), and BN a/b.
"""

import sys

import numpy as np

if "/opt/trn_rl_repo" not in sys.path:
    sys.path.insert(0, "/opt/trn_rl_repo")

B, C, N, M, OUT = 8, 128, 4096, 4096, 128
NJ = N // 128           # 32 n-chunks
MW = 1024               # m-window width (exp width; 2x512 matmul cols)
NMW = M // MW           # 8 m-windows
EPS_BN = 1e-5
NEG_SLOPE = 0.01


def _apply_bir_passes():
    """Ldweights dedup + single-wait legalization (same as baseline)."""
    import json

    import concourse.bass as bass

    if getattr(bass.Bass, "_bir_passes_applied", False):
        return
    orig = bass.Bass.to_json_bytes

    def patched(self):
        bir = json.loads(orig(self))
        for fn in bir.get("functions", []):
            for blk in fn.get("blocks", []):
                insts = blk.get("instructions", [])
                last_ldw = {}
                kept = []
                for ins in insts:
                    if ins.get("opcode") == "Ldweights":
                        eng = ins.get("engine")
                        key = json.dumps(
                            [
                                ins.get("ins"),
                                ins.get("perf_mode"),
                                ins.get("is_transpose"),
                                ins.get("tile_position"),
                            ],
                            sort_keys=True,
                        )
                        ow = (ins.get("sync_info") or {}).get("on_wait") or []
                        upd = (ins.get("sync_info") or {}).get("on_update") or []
                        if last_ldw.get(eng) == key and not upd:
                            if ow:
                                kept.append(
                                    {
                                        "debug": ins.get("debug", 0),
                                        "engine": eng,
                                        "ins": [],
                                        "name": ins["name"] + "-dedup",
                                        "opcode": "NoOp",
                                        "outs": [],
                                        "sync_info": {
                                            "on_update": [],
                                            "on_wait": ow,
                                        },
                                    }
                                )
                            continue
                        last_ldw[eng] = key
                    kept.append(ins)
                new_insts = []
                for ins in kept:
                    si = ins.get("sync_info")
                    ow = (si or {}).get("on_wait") or []
                    if len(ow) > 1:
                        for k, w in enumerate(ow[:-1]):
                            new_insts.append(
                                {
                                    "debug": ins.get("debug", 0),
                                    "engine": ins["engine"],
                                    "ins": [],
                                    "name": f"{ins['name']}-w{k}",
                                    "opcode": "NoOp",
                                    "outs": [],
                                    "sync_info": {
                                        "on_update": [],
                                        "on_wait": [w],
                                    },
                                }
                            )
                        si["on_wait"] = [ow[-1]]
                    new_insts.append(ins)
                blk["instructions"] = new_insts
        return json.dumps(bir).encode()

    bass.Bass.to_json_bytes = patched
    bass.Bass._bir_passes_applied = True


def _bcast(ap, parts):
    """Partition-stride-0 view of a [1, ...] DRAM AP, for DMA broadcast."""
    import concourse.bass as bass

    return bass.AP(
        tensor=ap.tensor,
        offset=ap.offset,
        ap=[[0, parts]] + [list(d) for d in ap.ap[1:]],
    )


def _ap3(sl, t_stride, t_n, m_stride, m_n):
    """3D AP view [partition][t][m] of a 2D tile slice (for DoubleRow)."""
    import concourse.bass as bass

    return bass.AP(
        tensor=sl.tensor,
        offset=sl.offset,
        ap=[list(sl.ap[0]), [t_stride, t_n], [m_stride, m_n]],
    )


def build_nc(repeats: int = 1, o3_mode: str = "dr", lrelu_mode: str = "dve",
             hints: bool = True):
    import concourse.bass as bass
    import concourse.tile as tile
    from concourse import bass_isa, mybir

    _apply_bir_passes()

    f32 = mybir.dt.float32
    bf16 = mybir.dt.bfloat16
    f8 = mybir.dt.float8e4
    ALU = mybir.AluOpType
    ACTF = mybir.ActivationFunctionType
    dr = o3_mode == "dr"
    xw_dt = f8 if dr else bf16
    pt_dt = f8 if dr else bf16

    nc = bass.Bass("TRN2")
    xn_d = nc.dram_tensor("xn", [C, N], bf16, kind="ExternalInput")
    gn_d = nc.dram_tensor("gn", [C, M], bf16, kind="ExternalInput")
    xw_d = nc.dram_tensor("xw", [128, NJ * OUT], xw_dt, kind="ExternalInput")
    ab_d = nc.dram_tensor("ab", [OUT, 2], f32, kind="ExternalInput")
    y_d = nc.dram_tensor("y", [OUT, M], bf16, kind="ExternalOutput")

    with tile.TileContext(nc) as tc:
        with (
            tc.tile_pool(name="const", bufs=1) as const,
            tc.tile_pool(name="sb", bufs=1) as sb,
            tc.tile_pool(name="rws", bufs=2) as rws,
            tc.tile_pool(name="ep", bufs=2) as ep,
            tc.tile_pool(name="drp", bufs=2, space="DRAM") as drp,
            tc.tile_pool(name="stp", bufs=2, space="PSUM") as stp,
            tc.tile_pool(name="o3p", bufs=1, space="PSUM") as o3p,
            tc.tile_pool(name="rwp", bufs=1, space="PSUM") as rwp,
        ):
            ab_sb = const.tile([OUT, 2], f32, tag="ab", name="ab_sb")
            nc.gpsimd.dma_start(out=ab_sb, in_=ab_d[:])
            ones_st = const.tile([128, 2], f8 if dr else bf16, tag="ones", name="ones_st")
            nc.vector.memset(ones_st, 1.0)
            ones_bc = const.tile([1, 128], f32, tag="onesbc", name="ones_bc")
            nc.vector.memset(ones_bc, 1.0)
            xn_sb = sb.tile([C, N], bf16, tag="xn", name="xn_sb")
            gn_sb = sb.tile([C, M], bf16, tag="gn", name="gn_sb")
            xw_sb = sb.tile([128, NJ * OUT], xw_dt, tag="xw", name="xw_sb")
            nc.gpsimd.dma_start(out=xn_sb, in_=xn_d[:])
            nc.gpsimd.dma_start(out=gn_sb, in_=gn_d[:])
            nc.gpsimd.dma_start(out=xw_sb, in_=xw_d[:])

            # output staging (dynamic DMA offsets don't compile; DVE writes
            # the window, one static DMA ships the full tensor at the end)
            y_all = sb.tile([OUT, M], bf16, tag="yall", name="y_all")
            # P^T for one m-window, n-chunk-major: pt[:, nj*MW + m]
            pt = sb.tile([128, NJ * MW], pt_dt, tag="pt", name="pt")

            def window_body(mw):
                o3 = o3p.tile([OUT, MW], f32, tag="o3", name="o3")
                rw_tiles = [
                    rwp.tile([1, 512], f32, tag=f"rw{h}", name=f"rowsum{h}")
                    for h in range(MW // 512)
                ]
                # stage the m-window of gn once: keeps the 32 S^T matmuls on
                # static APs (dynamic APs exhaust PE offset registers)
                gwin = ep.tile([C, MW], bf16, tag="gwin", name="gwin")
                nc.vector.tensor_copy(gwin, gn_sb[:, bass.ts(mw, MW)])
                for pj in range(NJ // 2):
                    for t in range(2):
                        nj = 2 * pj + t
                        st = stp.tile([128, MW], f32, tag="st", name="st")
                        for h in range(MW // 512):
                            nc.tensor.matmul(
                                st[:, h * 512 : (h + 1) * 512],
                                xn_sb[:, nj * 128 : (nj + 1) * 128],
                                gwin[:, h * 512 : (h + 1) * 512],
                                start=True,
                                stop=True,
                            )
                        nc.scalar.activation(
                            out=pt[:, nj * MW : (nj + 1) * MW],
                            in_=st,
                            func=ACTF.Exp,
                        )
                    if dr:
                        for h in range(MW // 512):
                            nc.tensor.matmul(
                                o3[:, h * 512 : (h + 1) * 512],
                                _ap3(
                                    xw_sb[:, pj * 256 : (pj + 1) * 256],
                                    128, 2, 1, 128,
                                ),
                                _ap3(pt[:, 2 * pj * MW + h * 512 :], MW, 2, 1, 512),
                                start=(pj == 0),
                                stop=(pj == NJ // 2 - 1),
                                perf_mode=mybir.MatmulPerfMode.DoubleRow,
                            )
                        # rows interleaved per pair: keeps the post-loop PE
                        # tail tiny so ACT never idles at the window boundary
                        for t in range(2):
                            nj = 2 * pj + t
                            for h in range(MW // 512):
                                nc.tensor.matmul(
                                    rw_tiles[h],
                                    ones_st[:, 0:1],
                                    pt[:, nj * MW + h * 512 : nj * MW + (h + 1) * 512],
                                    start=(nj == 0),
                                    stop=(nj == NJ - 1),
                                )
                    else:
                        for t in range(2):
                            nj = 2 * pj + t
                            for h in range(MW // 512):
                                nc.tensor.matmul(
                                    o3[:, h * 512 : (h + 1) * 512],
                                    xw_sb[:, nj * 128 : (nj + 1) * 128],
                                    pt[:, nj * MW + h * 512 : nj * MW + (h + 1) * 512],
                                    start=(nj == 0),
                                    stop=(nj == NJ - 1),
                                )

                # softmax denominator: ones-stationary matmuls over P^T
                # (consecutive, so the ones Ldweights dedups to one; DoubleRow
                # with a 1-row output miscompiles, so plain mode here)
                rr1 = rws.tile([1, MW], f32, tag="rr1", name="rr1")
                for h in range(MW // 512):
                    nc.vector.reciprocal(
                        out=rr1[:, h * 512 : (h + 1) * 512], in_=rw_tiles[h]
                    )
                # broadcast to 128 partitions via K=1 matmuls (DMA inside a
                # For_i body goes through the TriggerDma ISA path, which
                # miscompiles); output reuses a free st buffer, no extra PSUM
                rr = stp.tile([128, MW], f32, tag="st", name="rr")
                for h in range(MW // 512):
                    nc.tensor.matmul(
                        rr[:, h * 512 : (h + 1) * 512],
                        ones_bc[0:1, :],
                        rr1[:, h * 512 : (h + 1) * 512],
                        start=True,
                        stop=True,
                    )

                # epilogue: LeakyReLU, /rows, BN affine
                z = ep.tile([OUT, MW], f32, tag="z", name="z")
                if lrelu_mode == "act":
                    nc.scalar.activation(
                        out=z, in_=o3, func=ACTF.Lrelu, alpha=NEG_SLOPE
                    )
                else:
                    zt = ep.tile([OUT, MW], f32, tag="zt", name="zt")
                    nc.vector.tensor_scalar(
                        out=zt, in0=o3, scalar1=NEG_SLOPE, scalar2=None,
                        op0=ALU.mult,
                    )
                    nc.vector.tensor_tensor(out=z, in0=o3, in1=zt, op=ALU.max)
                del o3
                z2 = ep.tile([OUT, MW], f32, tag="z2", name="z2")
                nc.vector.tensor_tensor(out=z2, in0=z, in1=rr, op=ALU.mult)
                nc.vector.tensor_scalar(
                    out=y_all[:, bass.ts(mw, MW)],
                    in0=z2,
                    scalar1=ab_sb[:, 0:1],
                    scalar2=ab_sb[:, 1:2],
                    op0=ALU.mult,
                    op1=ALU.add,
                )

            hint_kw = (
                {"hint_engines": (mybir.EngineType.PE, mybir.EngineType.Activation)}
                if hints
                else {}
            )
            if repeats == 1:
                with tc.For_i(0, NMW, 1, **hint_kw) as mw:
                    window_body(mw)
            else:
                with tc.For_i(0, repeats, 1):
                    with tc.For_i(0, NMW, 1, **hint_kw) as mw:
                        window_body(mw)
            nc.gpsimd.dma_start(out=y_d[:], in_=y_all)
    return nc


_nc_cache: dict = {}


def _prep(input, target_g, weight, gamma, beta, running_mean, running_var):
    import ml_dtypes

    x = np.asarray(input, dtype=np.float32)
    g = np.asarray(target_g, dtype=np.float32)
    w = np.asarray(weight, dtype=np.float32)
    gamma = np.asarray(gamma, dtype=np.float32).reshape(OUT)
    beta = np.asarray(beta, dtype=np.float32).reshape(OUT)
    mean = np.asarray(running_mean, dtype=np.float32).reshape(OUT)
    var = np.asarray(running_var, dtype=np.float32).reshape(OUT)

    a_sc = (gamma / np.sqrt(var + EPS_BN)).astype(np.float32)
    b_sc = (beta - mean * a_sc).astype(np.float32)
    ab = np.ascontiguousarray(np.stack([a_sc, b_sc], axis=1))

    xn = x / np.maximum(np.sqrt((x * x).sum(axis=1, keepdims=True)), 1e-12)
    gn = g / np.maximum(np.sqrt((g * g).sum(axis=1, keepdims=True)), 1e-12)
    xn16 = np.ascontiguousarray(xn.astype(ml_dtypes.bfloat16))
    gn16 = np.ascontiguousarray(gn.astype(ml_dtypes.bfloat16))

    # xw[b, p, nj*128+o] = (x[b]^T @ W)[nj*128+p, o]
    xw = np.einsum("bcn,co->bno", x, w)
    xw = xw.reshape(B, NJ, 128, OUT).transpose(0, 2, 1, 3).reshape(B, 128, NJ * OUT)
    xw8 = np.ascontiguousarray(
        np.clip(xw, -224.0, 224.0).astype(ml_dtypes.float8_e4m3)
    )
    return [
        {"xn": xn16[b], "gn": gn16[b], "xw": xw8[b], "ab": ab} for b in range(B)
    ]


def kernel(input, target_g, weight, gamma, beta, running_mean, running_var):
    from concourse.bass_utils import run_bass_kernel_spmd

    if "nc" not in _nc_cache:
        _nc_cache["nc"] = build_nc(repeats=1)
    nc = _nc_cache["nc"]
    in_maps = _prep(
        input, target_g, weight, gamma, beta, running_mean, running_var
    )
    res = run_bass_kernel_spmd(nc, in_maps, core_ids=list(range(B)))
    return np.stack([res.results[b]["y"] for b in range(B)]).astype(np.float32)
